# revision 1
# baseline (speedup 1.0000x reference)
"""Bidirectional LSTM kernel for Trainium2 (Bass/Tile), B=64 S=256 I=H=512.

8-core data-parallel version: core c runs direction c//4 (0=fwd, 1=bwd) on
batch quarter c%4 (B_local=16), with the baseline's transposed gates^T
layout plus:
- gate order [f, i, g, o] so the i-sigmoid (on the c-critical chain) fires
  earlier in the ACT queue; o stays last (shortest post-activation path).
- a ~5us contiguous warmup matmul burst + per-step junk-matmul tail filler
  so the PE HAM clock-gate un-throttles to 2.4 GHz and stays there (the
  un-throttle needs ~3.4us of sustained PE-busy; the natural B=16 step
  pattern never provides it).
- the final h = o*tanh(c) multiply split into k01/k23 halves so the next
  step's first recurrent matmuls (which only read h chunks 0-1) start one
  DVE-op earlier.
Per-step floor on this architecture: 64 recurrent LDW+MM pairs at the
~27ns/instruction issue floor (~1.75us) + the serial gate-activation tail
(~1.45us): ~3.2us/step.
"""

import numpy as np
import ml_dtypes

P = 128
B_FULL = 64     # full batch
NB = 4          # batch shards per direction
BL = B_FULL // NB  # local batch
HD = 512        # hidden dim
ID = 512        # input dim
KH = HD // P    # 4 k-chunks over h
KI = ID // P    # 4 k-chunks over x
M4 = 4 * HD // P  # 16 m-chunks over the 4*H gate dim; order [f, g, i, o]
S_FULL = 256
SWEEP_FULL = 16

_NC_CACHE = {}


def build(S=S_FULL, SWEEP=SWEEP_FULL, B=BL):
    """Build and bacc-compile the single-core LSTM program (local batch B)."""
    import concourse.bacc as bacc
    import concourse.mybir as mybir
    import concourse.tile as tile
    from concourse.tile import add_dep_helper
    from contextlib import ExitStack

    AF = mybir.ActivationFunctionType
    bf16 = mybir.dt.bfloat16
    f32 = mybir.dt.float32

    assert S % SWEEP == 0
    n_sweeps = S // SWEEP
    COLS = SWEEP * B              # columns per sweep window
    NCH = max(1, COLS // 512)     # 512-col chunks per window
    NCOL = COLS // NCH            # columns per chunk (<= 512)
    TPC = NCOL // B               # timesteps covered per chunk
    n_groups = NCH * M4           # (n, m) GEMM groups per window
    assert n_groups % SWEEP == 0 or SWEEP % n_groups == 0
    gps = max(1, n_groups // SWEEP)  # groups emitted per step

    nc = bacc.Bacc("TRN2", target_bir_lowering=False, debug=False, num_devices=8)

    xT = nc.dram_tensor("xT", (P, KI, S * B), bf16, kind="ExternalInput")
    wx = nc.dram_tensor("wx", (P, KI, M4, P), bf16, kind="ExternalInput")
    wh = nc.dram_tensor("wh", (P, KH, M4, P), bf16, kind="ExternalInput")
    bias = nc.dram_tensor("bias", (P, M4), f32, kind="ExternalInput")
    ident = nc.dram_tensor("ident", (P, P), bf16, kind="ExternalInput")
    hsT = nc.dram_tensor("hsT", (S, KH, P, B), bf16, kind="ExternalOutput")

    with tile.TileContext(nc) as tc, ExitStack() as ctx:
        constp = ctx.enter_context(tc.tile_pool(name="const", bufs=1))
        xinp = ctx.enter_context(tc.tile_pool(name="xin", bufs=3))
        ringp = ctx.enter_context(tc.tile_pool(name="ring", bufs=3))
        statep = ctx.enter_context(tc.tile_pool(name="state", bufs=4))
        ewp = ctx.enter_context(tc.tile_pool(name="ew", bufs=4))
        psg0 = ctx.enter_context(tc.tile_pool(name="psum_g0", bufs=1, space="PSUM"))
        psg1 = ctx.enter_context(tc.tile_pool(name="psum_g1", bufs=1, space="PSUM"))
        psg2 = ctx.enter_context(tc.tile_pool(name="psum_g2", bufs=1, space="PSUM"))
        psg3 = ctx.enter_context(tc.tile_pool(name="psum_g3", bufs=1, space="PSUM"))
        psx = ctx.enter_context(tc.tile_pool(name="psum_x", bufs=4, space="PSUM"))

        wx_sb = constp.tile([P, KI, M4, P], bf16)
        wh_sb = constp.tile([P, KH, M4, P], bf16)
        for k in range(KI):
            nc.sync.dma_start(out=wx_sb[:, k], in_=wx.ap()[:, k])
        for k in range(KH):
            nc.sync.dma_start(out=wh_sb[:, k], in_=wh.ap()[:, k])
        bias_sb = constp.tile([P, M4], f32)
        nc.sync.dma_start(out=bias_sb[:], in_=bias.ap())
        id_sb = constp.tile([P, P], bf16)
        nc.sync.dma_start(out=id_sb[:], in_=ident.ap())

        x_bufs = {}
        ring_bufs = {}

        def load_x(s):
            t_ = xinp.tile([P, KI, COLS], bf16, tag="xin", name=f"xin{s}")
            nc.sync.dma_start(out=t_[:], in_=xT.ap()[:, :, s * COLS:(s + 1) * COLS])
            x_bufs[s] = t_

        def new_ring(s):
            ring_bufs[s] = ringp.tile([P, SWEEP, M4, B], bf16, tag="ring", name=f"ring{s}")

        def sweep_group(s, n, m, after=None, evict_dve=False, evict_after=None):
            xb = x_bufs[s]
            rb = ring_bufs[s]
            pt = psx.tile([P, TPC, B], f32, tag="psx")
            last = None
            for k in range(KI):
                mm = nc.tensor.matmul(
                    pt[:], wx_sb[:, k, m, :], xb[:, k, n * NCOL:(n + 1) * NCOL],
                    start=(k == 0), stop=(k == KI - 1),
                )
                if k == 0 and after is not None:
                    add_dep_helper(mm.ins, after.ins, sync=False,
                                   reason="pin sweep into step tail")
                last = mm
            if evict_dve:
                ev = nc.vector.tensor_scalar_add(
                    out=rb[:, n * TPC:(n + 1) * TPC, m, :], in0=pt[:],
                    scalar1=bias_sb[:, m:m + 1],
                )
            else:
                ev = nc.scalar.activation(
                    rb[:, n * TPC:(n + 1) * TPC, m, :], pt[:],
                    AF.Identity, bias=bias_sb[:, m:m + 1],
                )
            if evict_after is not None:
                add_dep_helper(ev.ins, evict_after.ins, sync=False,
                               reason="evict after step chain ops")
            return last

        GW = NCH * M4
        total_groups = n_sweeps * GW
        PRO = min(total_groups, M4 + 4 * gps)

        def emit_gi(gi, after=None, evict_dve=False, evict_after=None):
            gs, rem = divmod(gi, GW)
            gn, gm = divmod(rem, M4)
            if rem == 0:
                load_x(gs)
                new_ring(gs)
            return sweep_group(gs, gn, gm, after=after, evict_dve=evict_dve,
                               evict_after=evict_after)

        for gi in range(PRO):
            emit_gi(gi)

        # HAM warmup: ~5us of contiguous junk matmuls so the PE clock-gate
        # un-throttles (needs ~3.4us sustained busy at 1.2 GHz); the steady
        # loop's gaps are short enough to stay warm afterwards.
        wt = psx.tile([P, TPC, B], f32, tag="psx", name="warm")
        warm_last = None
        for wi in range(24):
            wm = nc.tensor.matmul(
                wt[:], id_sb[:], wx_sb[:, 0, 0:2, :],
                start=True, stop=True)
            if warm_last is not None:
                add_dep_helper(wm.ins, warm_last.ins, sync=False,
                               reason="warmup chain")
            warm_last = wm

        h_prev = None
        c_prev = None
        prev_tct = None
        prev_hmul = None
        last_sweep_mm = None
        MH = M4 // 2
        next_gi = PRO
        for t in range(S):
            s, sl = divmod(t, SWEEP)
            rb = ring_bufs[s]
            gpf = psg0.tile([P, KH, B], f32, tag="gf")
            gpg = psg1.tile([P, KH, B], f32, tag="gg")
            gpi = psg2.tile([P, KH, B], f32, tag="gi")
            gpo = psg3.tile([P, KH, B], f32, tag="go")
            tiles4 = (gpf, gpi, gpg, gpo)

            def gp_slot(m):
                return tiles4[m // KH], m % KH, KH

            first_pre = nc.tensor.matmul(
                gpf[:], id_sb[:], rb[:, sl, 0:KH, :],
                start=True, stop=(t == 0))
            nc.tensor.matmul(gpi[:], id_sb[:], rb[:, sl, KH:2 * KH, :],
                             start=True, stop=(t == 0))
            if t < 2:
                nc.tensor.matmul(gpg[:], id_sb[:], rb[:, sl, MH:MH + KH, :],
                                 start=True, stop=(t == 0))
                nc.tensor.matmul(gpo[:], id_sb[:], rb[:, sl, MH + KH:M4, :],
                                 start=True, stop=(t == 0))
            else:
                i_pb = nc.scalar.copy(gpg[:], rb[:, sl, MH:MH + KH, :])
                i_pc = nc.vector.tensor_copy(out=gpo[:], in_=rb[:, sl, MH + KH:M4, :])
                if prev_tct is not None:
                    add_dep_helper(i_pb.ins, prev_tct.ins, sync=False,
                                   reason="preI after prev tct")
                    add_dep_helper(i_pc.ins, prev_hmul.ins, sync=False,
                                   reason="preO after prev h")
            if t == 0:
                add_dep_helper(first_pre.ins, warm_last.ins, sync=False,
                               reason="steps after warmup")
            if last_sweep_mm is not None:
                add_dep_helper(first_pre.ins, last_sweep_mm.ins, sync=False,
                               reason="preloads after prior step sweeps")
            last_h_mm = first_pre
            if t > 0:
                skip = t >= 2
                for m in range(M4):
                    gp_t, ml, nl = gp_slot(m)
                    for k in range(KH):
                        last_h_mm = nc.tensor.matmul(
                            gp_t[:, ml, :], wh_sb[:, k, m, :], h_prev[:, k, :],
                            start=False,
                            stop=(not (skip and m >= MH)
                                  and k == KH - 1 and ml == nl - 1),
                            skip_group_check=(skip and m >= MH))

            sf = ewp.tile([P, KH, B], bf16, tag="sf")
            i_sf = nc.scalar.activation(sf[:], gpf[:], AF.Sigmoid)
            if t > 0:
                t2 = ewp.tile([P, KH, B], bf16, tag="t2")
                nc.vector.tensor_mul(out=t2[:], in0=sf[:], in1=c_prev[:])
            si = ewp.tile([P, KH, B], bf16, tag="si")
            i_si = nc.scalar.activation(si[:], gpi[:], AF.Sigmoid)
            add_dep_helper(i_si.ins, i_sf.ins, sync=False, reason="act order")
            tg = ewp.tile([P, KH, B], bf16, tag="tg")
            i_tg = nc.scalar.activation(tg[:], gpg[:], AF.Tanh)
            add_dep_helper(i_tg.ins, i_si.ins, sync=False, reason="act order")
            t1 = ewp.tile([P, KH, B], bf16, tag="t1")
            nc.vector.tensor_mul(out=t1[:], in0=si[:], in1=tg[:])
            so = ewp.tile([P, KH, B], bf16, tag="so")
            i_so = nc.scalar.activation(so[:], gpo[:], AF.Sigmoid)
            add_dep_helper(i_so.ins, i_tg.ins, sync=False, reason="act order")

            c_new = statep.tile([P, KH, B], bf16, tag="c")
            if t == 0:
                nc.vector.tensor_copy(out=c_new[:], in_=t1[:])
            else:
                nc.vector.tensor_add(out=c_new[:], in0=t1[:], in1=t2[:])
            tct = ewp.tile([P, KH, B], bf16, tag="tct")
            tct_inst = nc.scalar.activation(tct[:], c_new[:], AF.Tanh)
            add_dep_helper(tct_inst.ins, i_so.ins, sync=False, reason="act order")
            h_new = statep.tile([P, KH, B], bf16, tag="hT")
            # split the final h multiply so the next step's k0/k1 matmuls can
            # start ~one DVE-op earlier (they only read h chunks 0-1)
            HH = KH // 2
            hmul_a = nc.vector.tensor_mul(
                out=h_new[:, 0:HH, :], in0=so[:, 0:HH, :], in1=tct[:, 0:HH, :])
            hmul_inst = nc.vector.tensor_mul(
                out=h_new[:, HH:KH, :], in0=so[:, HH:KH, :], in1=tct[:, HH:KH, :])
            add_dep_helper(hmul_inst.ins, hmul_a.ins, sync=False,
                           reason="h halves order")
            nc.sync.dma_start(out=hsT.ap()[t].rearrange("k p b -> p k b"), in_=h_new[:])

            h_prev, c_prev = h_new, c_new
            prev_tct, prev_hmul = tct_inst, hmul_inst

            if next_gi < total_groups:
                for j in range(gps):
                    if next_gi >= total_groups:
                        break
                    dve = (j % 2 == 0)
                    last_sweep_mm = emit_gi(
                        next_gi, after=last_h_mm, evict_dve=dve,
                        evict_after=(hmul_inst if dve else tct_inst))
                    next_gi += 1
            else:
                last_sweep_mm = None
            # Keep-warm filler: junk matmuls pinned into the step tail so the
            # HAM activity window never reads mostly-idle (denser early while
            # the pipeline ramps).
            njunk = 7
            prev_pe = last_sweep_mm or last_h_mm
            for _ in range(njunk):
                jm = nc.tensor.matmul(
                    wt[:], id_sb[:], wx_sb[:, 0, 0:2, :],
                    start=True, stop=True)
                add_dep_helper(jm.ins, prev_pe.ins, sync=False,
                               reason="junk in tail")
                prev_pe = jm
            last_sweep_mm = prev_pe

    nc.compile()
    return nc


def _get_nc(S, SWEEP, B=BL):
    key = (S, SWEEP, B)
    if key not in _NC_CACHE:
        _NC_CACHE[key] = build(S, SWEEP, B)
    return _NC_CACHE[key]


def prep_core_inputs(x, Wc, bc, Wi, bi, Wf, bf, Wo, bo, reverse):
    """Pack one direction's inputs into the kernel's layouts. x: (B, S, I) f32."""
    bft = ml_dtypes.bfloat16
    if reverse:
        x = x[:, ::-1, :]
    S = x.shape[1]
    B = x.shape[0]
    Wcat = np.concatenate([Wf, Wi, Wc, Wo], axis=1)      # (I+H, 4H), gate order [f,i,g,o]
    bcat = np.concatenate([bf, bi, bc, bo]).astype(np.float32)
    Wx, Wh = Wcat[:ID], Wcat[ID:]

    xT = (
        x.transpose(2, 1, 0)                  # (I, S, B)
        .reshape(KI, P, S * B)
        .transpose(1, 0, 2)                   # (P, KI, S*B)
    )
    wxp = Wx.reshape(KI, P, M4, P).transpose(1, 0, 2, 3)
    whp = Wh.reshape(KH, P, M4, P).transpose(1, 0, 2, 3)
    biasp = bcat.reshape(M4, P).T
    return {
        "xT": np.ascontiguousarray(xT).astype(bft),
        "wx": np.ascontiguousarray(wxp).astype(bft),
        "wh": np.ascontiguousarray(whp).astype(bft),
        "bias": np.ascontiguousarray(biasp),
        "ident": np.eye(P, dtype=bft),
    }


def run_lstm(x, Wi_f, bi_f, Wf_f, bf_f, Wc_f, bc_f, Wo_f, bo_f,
             Wi_b, bi_b, Wf_b, bf_b, Wc_b, bc_b, Wo_b, bo_b,
             trace=False, trace_cores=None):
    from concourse import bass_utils

    x = np.asarray(x, dtype=np.float32)
    S = x.shape[1]
    nc = _get_nc(S, SWEEP_FULL if S % SWEEP_FULL == 0 else S)
    ims = []
    for c in range(2 * NB):
        d = c // NB
        q = c % NB
        xq = x[q * BL:(q + 1) * BL]
        if d == 0:
            ims.append(prep_core_inputs(
                xq, Wc_f, bc_f, Wi_f, bi_f, Wf_f, bf_f, Wo_f, bo_f, False))
        else:
            ims.append(prep_core_inputs(
                xq, Wc_b, bc_b, Wi_b, bi_b, Wf_b, bf_b, Wo_b, bo_b, True))
    res = bass_utils.run_bass_kernel_spmd(
        nc, ims, core_ids=list(range(2 * NB)), trace=trace, trace_cores=trace_cores,
    )
    outs = []
    for c in range(2 * NB):
        hs = res.results[c]["hsT"].astype(np.float32)   # (S, KH, P, BL)
        if c // NB == 1:
            hs = hs[::-1]
        outs.append(hs.transpose(0, 3, 1, 2).reshape(S, BL, HD))  # (S, BL, H)
    fwd = np.concatenate(outs[:NB], axis=1)   # (S, B, H)
    bwd = np.concatenate(outs[NB:], axis=1)
    out = np.concatenate([fwd, bwd], axis=2).transpose(1, 0, 2)  # (B, S, 2H)
    return np.ascontiguousarray(out), res


def kernel(x, Wi_f, bi_f, Wf_f, bf_f, Wc_f, bc_f, Wo_f, bo_f,
           Wi_b, bi_b, Wf_b, bf_b, Wc_b, bc_b, Wo_b, bo_b):
    out, _ = run_lstm(x, Wi_f, bi_f, Wf_f, bf_f, Wc_f, bc_f, Wo_f, bo_f,
                      Wi_b, bi_b, Wf_b, bf_b, Wc_b, bc_b, Wo_b, bo_b)
    return out

